# revision 37
# baseline (speedup 1.0000x reference)
"""Multi-head causal attention (B=4, T=2048, D=512, H=8) on 8 TRN2 NeuronCores.

Sharding: core c handles batch b = c//2 and head-group hg = c%2 (4 heads,
256 output dims).  No collectives needed — 8 fully independent problems.

Per-core algorithm (matmul inputs bf16, O^T accumulation f32 in PSUM):
  - host passes x^T (D,T) and W^T slices (D, 256) in bf16 + a [128,128]
    triangular causal mask
  - Q^T,K^T projections:  qT[dh2,T] = W2h @ xT, two heads stacked per tile
    (head 2g at partitions 0-63, head 2g+1 at partitions 64-127)
  - V projection into augmented-V tiles [k-tile 128, 65] (ones column
    appended -> the O^T matmul also produces the softmax denominator row).
    The ones-column is OPTIMAL for the denominator: any scheme that reads
    the 8.9M-element P matrix on another engine (DVE/gpsimd adds for
    col-packed M=64+64 AV) costs >= the AV-packing saving.
  - flash-style over head-PAIRS: for each (q-block, pair g), per k-tile the
    two heads' score matmuls S^T[k,q] = K^T.T @ Q^T are row-tiled
    (tile_position (0,0)/(64,0) via base_partition auto-derive) and stream
    CONCURRENTLY on the PE (measured dstart 3ns) — K=64 each, so the pair
    costs one matmul's stream time.  exp via one ACT instr per k-tile over
    both heads [128, 2, width] (scale=1/8 folded; no max subtraction:
    |scores| < ~4); causal via per-k-tile width restriction + triangle-mask
    multiply on the boundary block (the in-block triangle waste lives in
    the partition dim and is free on every engine).
  - O^T accumulated in PSUM over k-tiles (start/stop groups), software
    pipelined TWO batches deep (AV(kt) emitted after scores(kt+2)) so the
    PE rides through exp latency AND through the ot-slot WAR at unit entry.
  - epilogue per unit: O^T+denominator cast to bf16 in column HALVES —
    cols [0,256) are final right after diagonal kt qb*4+1, so most of the
    cast happens mid-unit and the single ot PSUM slot frees early — then
    DMA'd out UNNORMALIZED; the host divides by the denominator row and
    transposes (removes all PE transposes + DVE normalize work).

Scheduling (program order == Tile priority): 12 warm matmuls burn the
x-block-0 DMA shadow (HAM ramp) + 2-warm bridge over the wq wait; then
units run qb ASCENDING (unit (0,g0) needs only x block 0 for K chunk 0,
Q chunk 0 and V tiles 0-3, so the exp stream starts as soon as the first
quarter of x^T lands), g=0 then 1, with later projections woven between
batches as PE filler in x-arrival order.  The last three units have no
fillers left; their first projections are PREFILLED before the unit's
first scores (in-order PE queue: a filler emitted behind st-WAR-blocked
scores is itself stalled).  x^T is DMA'd as 16 column-block pieces over
the SP/ACT/gpsimd DGE queues, ACT-queue pieces all landing before the
first exp; outputs stream per unit on SP (last unit split SP/ACT).

PSUM budget (8 banks x 2KB): st [128,2,512] f32 x2 bufs (4) + ot
[128,2,512] x1 (2) + proj ps [128,512] x2 (2).

Measured on this container's device (PE ~2.45GHz warm): 110.2-110.9us
over 10 reps, rel_err 3.8e-3.  The device drifts between ~2.45GHz and a
~2.0GHz P0 power state run-to-run (~+18% exec when hot) — compare runs
via the score-MM median duration (372ns warm vs 446ns hot), not raw ns.
Fixed overheads outside kernel control: ~7.2us preamble (barrier rings +
ACT table load before any DMA trigger), ~6.5us teardown semaphore chain.
"""

import numpy as np
import ml_dtypes

T = 2048
D = 512
HG = 4  # heads per core
DH = 64
OUTW = HG * DH  # 256
QB = 512  # q block (columns of S^T tiles)
NQB = T // QB  # 4
NKT = T // 128  # 16 k-tiles
N_CORES = 8

_CACHE = {}


def _build_nc():
    import concourse.bacc as bacc
    import concourse.tile as tile
    import concourse.mybir as mybir
    from contextlib import ExitStack

    fp32 = mybir.dt.float32
    bf16 = mybir.dt.bfloat16
    EXP = mybir.ActivationFunctionType.Exp

    nc = bacc.Bacc(None, target_bir_lowering=False)

    xt_d = nc.declare_dram_parameter("xt", [D, T], bf16, isOutput=False)
    wqt_d = nc.declare_dram_parameter("wqt", [D, OUTW], bf16, isOutput=False)
    wkt_d = nc.declare_dram_parameter("wkt", [D, OUTW], bf16, isOutput=False)
    wvt_d = nc.declare_dram_parameter("wvt", [D, OUTW], bf16, isOutput=False)
    cmask_d = nc.declare_dram_parameter("cmask", [128, 128], bf16, isOutput=False)
    # unnormalized O^T + denominator row: [head, 65, qb, 512]
    out_d = nc.declare_dram_parameter("out", [HG, 65, NQB, QB], bf16, isOutput=True)

    with tile.TileContext(nc) as tc, ExitStack() as ctx:
        const = ctx.enter_context(tc.tile_pool(name="const", bufs=1))
        ps_s = ctx.enter_context(tc.tile_pool(name="ps_s", bufs=2, space="PSUM"))
        pt_pool = ctx.enter_context(tc.tile_pool(name="pt", bufs=4))
        osb_pool = ctx.enter_context(tc.tile_pool(name="osb", bufs=2))

        # ---- input loads ----
        # x^T arrives as 16 column-block pieces (chunk c x q-block b), DMA'd
        # in ascending consumption order (block 0 gates the prologue
        # projections) and spread over the three DGE queues.  The scalar
        # (ACT) queue only carries pieces that finish BEFORE the first exp,
        # so triggers never steal ACT time from the exp stream.
        xTb = [[const.tile([128, QB], bf16, tag=f"xT{c}_{b}", name=f"xT{c}_{b}")
                for b in range(4)] for c in range(4)]
        wkT = [const.tile([128, OUTW], bf16, tag=f"wkT{c}", name=f"wkT{c}")
               for c in range(4)]
        wqT = [const.tile([128, OUTW], bf16, tag=f"wqT{c}", name=f"wqT{c}")
               for c in range(4)]
        wvT = [const.tile([128, OUTW], bf16, tag=f"wvT{c}", name=f"wvT{c}")
               for c in range(4)]
        mask_sb = const.tile([128, 128], bf16, name="mask_sb")

        def ld_w(eng, wt, dram, c):
            eng.dma_start(out=wt[c][:], in_=dram[c * 128:(c + 1) * 128, :])

        def ld_x(eng, c, b):
            eng.dma_start(
                out=xTb[c][b][:],
                in_=xt_d[c * 128:(c + 1) * 128, b * QB:(b + 1) * QB],
            )

        ld_w(nc.sync, wkT, wkt_d, 0)
        ld_w(nc.sync, wkT, wkt_d, 1)
        ld_w(nc.sync, wkT, wkt_d, 3)
        ld_x(nc.sync, 0, 0)
        ld_w(nc.sync, wqT, wqt_d, 0)
        ld_w(nc.sync, wqT, wqt_d, 1)
        ld_x(nc.sync, 0, 1)
        ld_x(nc.sync, 1, 1)
        ld_x(nc.sync, 2, 1)
        ld_x(nc.sync, 3, 1)
        ld_x(nc.sync, 0, 3)
        ld_x(nc.sync, 1, 3)

        ld_w(nc.scalar, wkT, wkt_d, 2)
        ld_x(nc.scalar, 2, 0)
        ld_x(nc.scalar, 1, 0)
        ld_w(nc.scalar, wqT, wqt_d, 2)
        ld_w(nc.scalar, wqT, wqt_d, 3)
        nc.scalar.dma_start(out=mask_sb[:], in_=cmask_d[:])

        # slow SWDGE triggers (~1us each) only for the one block-0 piece
        # that balances the fast queues, plus late-needed pieces
        ld_x(nc.gpsimd, 3, 0)
        for c in range(4):
            ld_w(nc.gpsimd, wvT, wvt_d, c)
        ld_x(nc.gpsimd, 0, 2)
        ld_x(nc.gpsimd, 1, 2)
        ld_x(nc.gpsimd, 2, 2)
        ld_x(nc.gpsimd, 3, 2)
        ld_x(nc.gpsimd, 2, 3)
        ld_x(nc.gpsimd, 3, 3)

        # ---- HAM warm-up burst ----
        # The PE clock needs a fully-busy window to ramp.  Burn the x^T DMA
        # shadow with dense dummy matmuls so real work starts warm.
        warm_w = const.tile([128, 128], bf16, name="warm_w")
        warm_x = const.tile([128, QB], bf16, name="warm_x")
        nc.vector.memset(warm_w[:], 0.5)
        nc.vector.memset(warm_x[:], 0.5)
        # 12 warms ≈ the ~6.5us wk/wq/x-block-0 DMA window (cold 512ns each,
        # ~256ns once the ramp trips mid-burst) — sized so the PE never idles
        # a full MID window before the first projection.
        warm_ps = ps_s.tile([128, QB], fp32, tag="ps", name="warm_ps")
        for _ in range(12):
            nc.tensor.matmul(warm_ps[:], warm_w[:], warm_x[:], start=True, stop=True)

        # ---- persistent SBUF tensors ----
        qT = [const.tile([128, T], bf16, tag=f"qT{g}", name=f"qT{g}") for g in range(2)]
        kT = [const.tile([128, T], bf16, tag=f"kT{g}", name=f"kT{g}") for g in range(2)]
        vaug = const.tile([128, NKT, HG, 65], bf16, name="vaug")
        nc.vector.memset(vaug[:, :, :, 64:65], 1.0)

        def proj_qk(dst, wt, g, qb4, c0=0, c1=QB):
            ps = ps_s.tile([128, c1 - c0], fp32, tag="ps", name="ps")
            for c in range(4):
                nc.tensor.matmul(
                    ps[:],
                    wt[c][:, g * 128:(g + 1) * 128],
                    xTb[c][qb4][:, c0:c1],
                    start=(c == 0),
                    stop=(c == 3),
                )
            nc.vector.tensor_copy(
                dst[g][:, qb4 * QB + c0:qb4 * QB + c1], ps[:])

        def proj_v(tt):
            ps = ps_s.tile([128, OUTW], fp32, tag="ps", name="ps")
            for c in range(4):
                nc.tensor.matmul(
                    ps[:],
                    xTb[c][tt // 4][:, (tt % 4) * 128:(tt % 4 + 1) * 128],
                    wvT[c][:, 0:OUTW],
                    start=(c == 0),
                    stop=(c == 3),
                )
            nc.vector.tensor_copy(
                vaug[:, tt, :, 0:64],
                ps[:].rearrange("p (h d) -> p h d", h=HG),
            )

        def attn_unit(qb, g, fillers=None, prefill=None, last=False):
            """One (q-block, head-pair) attention unit.  The two heads'
            score matmuls per k-tile are row-tiled (partitions 0-63 /
            64-127) and run concurrently on the PE.  AV(kt-1) is emitted
            after scores(kt) (1-deep software pipeline) so the PE works
            through the exp latency.  fillers[j] = list of closures run
            as PE filler after batch j's scores."""
            nkt = qb * 4 + 4
            # prefill runs BEFORE the first scores: at a unit boundary the
            # first scores block on the st-slot WAR until ACT drains the
            # previous unit's diagonal-exp backlog, and the in-order PE
            # queue would stall fillers emitted behind them.
            if prefill:
                for f in prefill:
                    f()
            ot = ps_s.tile([128, 2, QB], fp32, tag="ot", bufs=1, name="ot")
            osb = osb_pool.tile([65, 2, QB], bf16, tag="osb", name="osb")
            pend = []  # (kt, pt, q0, width) awaiting AV matmuls (lag 2)

            def emit_av(kt, pt, q0, width):
                for i in range(2):
                    nc.tensor.matmul(
                        ot[0:65, i, q0:q0 + width],
                        vaug[:, kt, 2 * g + i, :],
                        pt[:, i, q0:q0 + width],
                        start=(kt == 0),
                        stop=(kt == nkt - 1),
                    )
                # O^T columns [0,256) are final after diagonal kt qb*4+1,
                # columns [256,512) after the last kt: cast each half as
                # soon as it is final (subtile deps) so most of the cast is
                # off the unit boundary and the ot PSUM slot frees early.
                if kt == qb * 4 + 1:
                    nc.vector.tensor_copy(osb[:, :, 0:256], ot[0:65, :, 0:256])
                    if last:
                        # ship the final unit's first half early so the
                        # kernel tail only transfers 256 columns per head
                        for i in range(2):
                            nc.sync.dma_start(
                                out=out_d[2 * g + i, :, qb, 0:256],
                                in_=osb[:, i, 0:256],
                            )
                elif kt == nkt - 1:
                    nc.vector.tensor_copy(osb[:, :, 256:QB],
                                          ot[0:65, :, 256:QB])

            for kt in range(nkt):
                diag = kt >= qb * 4
                q0 = (kt - qb * 4) * 128 if diag else 0
                width = QB - q0
                st = ps_s.tile([128, 2, QB], fp32, tag="st", name="st")
                for i in range(2):
                    nc.tensor.matmul(
                        st[:, i, q0:q0 + width],
                        kT[g][64 * i:64 * i + 64, kt * 128:(kt + 1) * 128],
                        qT[g][64 * i:64 * i + 64,
                              qb * QB + q0:qb * QB + q0 + width],
                        start=True,
                        stop=True,
                    )
                if fillers:
                    for f in fillers.get(kt, ()):
                        f()
                # AV lags 2 batches so the unit's first AV (which waits on
                # the previous unit's ot WAR) issues behind two score pairs.
                if len(pend) == 2:
                    emit_av(*pend.pop(0))
                pt = pt_pool.tile([128, 2, QB], bf16, tag="pt", name="pt")
                nc.scalar.activation(
                    pt[:, :, q0:q0 + width], st[:, :, q0:q0 + width],
                    func=EXP, scale=0.125,
                )
                if diag:
                    for i in range(2):
                        nc.vector.tensor_mul(
                            pt[:, i, q0:q0 + 128], pt[:, i, q0:q0 + 128],
                            mask_sb[:],
                        )
                pend.append((kt, pt, q0, width))
            for p in pend:
                emit_av(*p)

            # stream out unnormalized O^T + denominator row (host divides
            # + transposes); the casts already happened in emit_av.
            lo = 256 if last else 0  # last unit's first half went out early
            nc.sync.dma_start(out=out_d[2 * g, :, qb, lo:QB],
                              in_=osb[:, 0, lo:QB])
            eng = nc.scalar if last else nc.sync
            eng.dma_start(out=out_d[2 * g + 1, :, qb, lo:QB],
                          in_=osb[:, 1, lo:QB])

        # ---- schedule ----
        # qb ASCENDING: unit (0, g0) needs only x block 0 (K chunk 0, Q
        # chunk 0, V tiles 0-3), so the exp stream starts as soon as the
        # first quarter of x^T lands.  Each unit's fillers project what the
        # NEXT units need, in x-arrival order.  An 8-warm bridge covers the
        # wq DMA wait.
        # NOTE: a narrow (N=128) first K projection starts the exp stream
        # earlier on paper but thins the PE right at the HAM window edge —
        # measured: MID re-throttle at ~16us and a HALF-CLOCK first unit.
        # Keep the prologue dense.
        proj_qk(kT, wkT, 0, 0)
        for _ in range(2):
            nc.tensor.matmul(warm_ps[:], warm_w[:], warm_x[:], start=True, stop=True)
        proj_qk(qT, wqT, 0, 0)

        def F(*items):
            out = []
            for it in items:
                if it[0] == "v":
                    out.append(lambda t=it[1]: proj_v(t))
                elif it[0] == "k":
                    out.append(lambda g=it[1], c=it[2]: proj_qk(kT, wkT, g, c))
                else:
                    out.append(lambda g=it[1], c=it[2]: proj_qk(qT, wqT, g, c))
            return out

        fill_00 = {
            0: F(("v", 0)),
            1: F(("v", 1)),
            2: F(("v", 2), ("k", 1, 0)),
            3: F(("v", 3), ("q", 1, 0)),
        }
        fill_01 = {
            1: F(("k", 0, 1)),
            2: F(("q", 0, 1)),
            3: F(("v", 4)),
        }
        fill_10 = {
            0: F(("v", 5)),
            1: F(("k", 1, 1)),
            2: F(("q", 1, 1)),
            3: F(("v", 6)),
            4: F(("v", 7)),
            5: F(("k", 0, 2)),
            6: F(("v", 8)),
            7: F(("v", 9)),
        }
        fill_11 = {
            0: F(("q", 0, 2)),
            1: F(("v", 10)),
            2: F(("v", 11)),
            4: F(("q", 1, 2)),
            5: F(("k", 0, 3)),
            6: F(("v", 12)),
            7: F(("v", 13)),
        }
        fill_20 = {
            0: F(("v", 14)),
            1: F(("v", 15)),
            2: F(("q", 0, 3)),
        }
        attn_unit(0, 0, fillers=fill_00)
        attn_unit(0, 1, fillers=fill_01)
        attn_unit(1, 0, fillers=fill_10)
        attn_unit(1, 1, fillers=fill_11)
        attn_unit(2, 0, fillers=fill_20)
        # the last projections are saved as PREFILL for the otherwise
        # fillerless late units (see attn_unit)
        attn_unit(2, 1, prefill=F(("k", 1, 2)))
        attn_unit(3, 0, prefill=F(("k", 1, 3)))
        attn_unit(3, 1, prefill=F(("q", 1, 3)), last=True)

    nc.finalize()
    return nc


def _get_nc():
    if "nc" not in _CACHE:
        _CACHE["nc"] = _build_nc()
    return _CACHE["nc"]


def _make_cmask():
    # triangle: mask[p, f] = 1.0 iff p <= f
    p = np.arange(128)[:, None]
    f = np.arange(128)[None, :]
    return (p <= f).astype(ml_dtypes.bfloat16)


def _make_in_maps(x, Wq, Wk, Wv):
    bf = ml_dtypes.bfloat16
    cmask = _make_cmask()
    in_maps = []
    for c in range(N_CORES):
        b, hg = c // 2, c % 2
        r0 = hg * OUTW
        in_maps.append({
            "xt": np.ascontiguousarray(x[b].T).astype(bf),
            "wqt": np.ascontiguousarray(Wq[r0:r0 + OUTW].T).astype(bf),
            "wkt": np.ascontiguousarray(Wk[r0:r0 + OUTW].T).astype(bf),
            "wvt": np.ascontiguousarray(Wv[r0:r0 + OUTW].T).astype(bf),
            "cmask": cmask,
        })
    return in_maps


def _postprocess(results, B):
    """Host-side unshard: divide unnormalized O^T by the denominator row
    and transpose to natural [T, D] layout."""
    out = np.empty((B, T, D), dtype=np.float32)
    for c in range(N_CORES):
        b, hg = c // 2, c % 2
        r = results[c]["out"].astype(np.float32)  # [HG, 65, NQB, QB]
        r = r.reshape(HG, 65, T)
        for h in range(HG):
            blk = r[h, :64] / r[h, 64:65]  # [64, T]
            out[b, :, hg * OUTW + h * DH:hg * OUTW + (h + 1) * DH] = blk.T
    return out


def kernel(x, Wq, Wk, Wv):
    from concourse.bass_utils import run_bass_kernel_spmd

    nc = _get_nc()
    in_maps = _make_in_maps(x, Wq, Wk, Wv)
    res = run_bass_kernel_spmd(nc, in_maps, core_ids=list(range(N_CORES)))
    return _postprocess(res.results, x.shape[0])


# revision 39
# speedup vs baseline: 1.1607x; 1.1607x over previous
"""Multi-head causal attention (B=4, T=2048, D=512, H=8) on 8 TRN2 NeuronCores.

Sharding: core c handles batch b = c//2 and head-group hg = c%2 (4 heads,
256 output dims).  No collectives needed — 8 fully independent problems.

Per-core algorithm (matmul inputs bf16, O^T accumulation f32 in PSUM):
  - host passes x^T (D,T) and W^T slices (D, 256) in bf16 + a [128,128]
    triangular causal mask
  - Q^T,K^T projections:  qT[dh2,T] = W2h @ xT, two heads stacked per tile
    (head 2g at partitions 0-63, head 2g+1 at partitions 64-127)
  - V projection into augmented-V tiles [k-tile 128, 65] (ones column
    appended -> the O^T matmul also produces the softmax denominator row).
    The ones-column is OPTIMAL for the denominator: any scheme that reads
    the 8.9M-element P matrix on another engine (DVE/gpsimd adds for
    col-packed M=64+64 AV) costs >= the AV-packing saving.
  - flash-style over head-PAIRS: for each (q-block, pair g), per k-tile the
    two heads' score matmuls S^T[k,q] = K^T.T @ Q^T are row-tiled
    (tile_position (0,0)/(64,0) via base_partition auto-derive) and stream
    CONCURRENTLY on the PE (measured dstart 3ns) — K=64 each, so the pair
    costs one matmul's stream time.  exp via one ACT instr per k-tile over
    both heads [128, 2, width] (scale=1/8 folded; no max subtraction:
    |scores| < ~4); causal via per-k-tile width restriction + triangle-mask
    multiply on the boundary block (the in-block triangle waste lives in
    the partition dim and is free on every engine).
  - O^T accumulated in PSUM over k-tiles (start/stop groups), software
    pipelined TWO batches deep (AV(kt) emitted after scores(kt+2)) so the
    PE rides through exp latency AND through the ot-slot WAR at unit entry.
  - epilogue per unit: O^T+denominator cast to bf16 in column HALVES —
    cols [0,256) are final right after diagonal kt qb*4+1, so most of the
    cast happens mid-unit and the single ot PSUM slot frees early — then
    DMA'd out UNNORMALIZED; the host divides by the denominator row and
    transposes (removes all PE transposes + DVE normalize work).

Scheduling (program order == Tile priority): 12 warm matmuls burn the
x-block-0 DMA shadow (HAM ramp) + 2-warm bridge over the wq wait; then
units run qb ASCENDING (unit (0,g0) needs only x block 0 for K chunk 0,
Q chunk 0 and V tiles 0-3, so the exp stream starts as soon as the first
quarter of x^T lands), g=0 then 1, with later projections woven between
batches as PE filler in x-arrival order.  The last three units have no
fillers left; their first projections are PREFILLED before the unit's
first scores (in-order PE queue: a filler emitted behind st-WAR-blocked
scores is itself stalled).  x^T is DMA'd as 16 column-block pieces over
the SP/ACT/gpsimd DGE queues, ACT-queue pieces all landing before the
first exp; outputs stream per unit on SP (last unit split SP/ACT).

PSUM budget (8 banks x 2KB): st [128,2,512] f32 x2 bufs (4) + ot
[128,2,512] x1 (2) + proj ps [128,512] x2 (2).

Measured on this container's device (PE ~2.45GHz warm): 110.2-110.9us
over 10 reps, rel_err 3.8e-3.  The device drifts between ~2.45GHz and a
~2.0GHz P0 power state run-to-run (~+18% exec when hot) — compare runs
via the score-MM median duration (372ns warm vs 446ns hot), not raw ns.
Fixed overheads outside kernel control: ~7.2us preamble (barrier rings +
ACT table load before any DMA trigger), ~6.5us teardown semaphore chain.
"""

import numpy as np
import ml_dtypes

T = 2048
D = 512
HG = 4  # heads per core
DH = 64
OUTW = HG * DH  # 256
QB = 512  # q block (columns of S^T tiles)
NQB = T // QB  # 4
NKT = T // 128  # 16 k-tiles
N_CORES = 8

_CACHE = {}


def _build_nc():
    import concourse.bacc as bacc
    import concourse.tile as tile
    import concourse.mybir as mybir
    from contextlib import ExitStack

    fp32 = mybir.dt.float32
    bf16 = mybir.dt.bfloat16
    EXP = mybir.ActivationFunctionType.Exp

    nc = bacc.Bacc(None, target_bir_lowering=False)

    xt_d = nc.declare_dram_parameter("xt", [D, T], bf16, isOutput=False)
    wqt_d = nc.declare_dram_parameter("wqt", [D, OUTW], bf16, isOutput=False)
    wkt_d = nc.declare_dram_parameter("wkt", [D, OUTW], bf16, isOutput=False)
    wvt_d = nc.declare_dram_parameter("wvt", [D, OUTW], bf16, isOutput=False)
    cmask_d = nc.declare_dram_parameter("cmask", [128, 128], bf16, isOutput=False)
    # unnormalized O^T + denominator row: [head, 65, qb, 512]
    out_d = nc.declare_dram_parameter("out", [HG, 65, NQB, QB], bf16, isOutput=True)

    with tile.TileContext(nc) as tc, ExitStack() as ctx:
        const = ctx.enter_context(tc.tile_pool(name="const", bufs=1))
        ps_s = ctx.enter_context(tc.tile_pool(name="ps_s", bufs=2, space="PSUM"))
        pt_pool = ctx.enter_context(tc.tile_pool(name="pt", bufs=4))
        osb_pool = ctx.enter_context(tc.tile_pool(name="osb", bufs=2))

        # ---- input loads ----
        # x^T arrives as 16 column-block pieces (chunk c x q-block b), DMA'd
        # in ascending consumption order (block 0 gates the prologue
        # projections) and spread over the three DGE queues.  The scalar
        # (ACT) queue only carries pieces that finish BEFORE the first exp,
        # so triggers never steal ACT time from the exp stream.
        xTb = [[const.tile([128, QB], bf16, tag=f"xT{c}_{b}", name=f"xT{c}_{b}")
                for b in range(4)] for c in range(4)]
        wkT = [const.tile([128, OUTW], bf16, tag=f"wkT{c}", name=f"wkT{c}")
               for c in range(4)]
        wqT = [const.tile([128, OUTW], bf16, tag=f"wqT{c}", name=f"wqT{c}")
               for c in range(4)]
        wvT = [const.tile([128, OUTW], bf16, tag=f"wvT{c}", name=f"wvT{c}")
               for c in range(4)]
        mask_sb = const.tile([128, 128], bf16, name="mask_sb")

        def ld_w(eng, wt, dram, c):
            eng.dma_start(out=wt[c][:], in_=dram[c * 128:(c + 1) * 128, :])

        def ld_x(eng, c, b):
            eng.dma_start(
                out=xTb[c][b][:],
                in_=xt_d[c * 128:(c + 1) * 128, b * QB:(b + 1) * QB],
            )

        ld_w(nc.sync, wkT, wkt_d, 0)
        ld_w(nc.sync, wkT, wkt_d, 1)
        ld_w(nc.sync, wkT, wkt_d, 3)
        ld_x(nc.sync, 0, 0)
        ld_w(nc.sync, wqT, wqt_d, 0)
        ld_w(nc.sync, wqT, wqt_d, 1)
        ld_x(nc.sync, 0, 1)
        ld_x(nc.sync, 1, 1)
        ld_x(nc.sync, 2, 1)
        ld_x(nc.sync, 3, 1)
        ld_x(nc.sync, 0, 3)
        ld_x(nc.sync, 1, 3)

        ld_w(nc.scalar, wkT, wkt_d, 2)
        ld_x(nc.scalar, 2, 0)
        ld_x(nc.scalar, 1, 0)
        ld_w(nc.scalar, wqT, wqt_d, 2)
        ld_w(nc.scalar, wqT, wqt_d, 3)
        nc.scalar.dma_start(out=mask_sb[:], in_=cmask_d[:])

        # slow SWDGE triggers (~1us each) only for the one block-0 piece
        # that balances the fast queues, plus late-needed pieces
        ld_x(nc.gpsimd, 3, 0)
        for c in range(4):
            ld_w(nc.gpsimd, wvT, wvt_d, c)
        ld_x(nc.gpsimd, 0, 2)
        ld_x(nc.gpsimd, 1, 2)
        ld_x(nc.gpsimd, 2, 2)
        ld_x(nc.gpsimd, 3, 2)
        ld_x(nc.gpsimd, 2, 3)
        ld_x(nc.gpsimd, 3, 3)

        # ---- HAM warm-up burst ----
        # The PE clock needs a fully-busy window to ramp.  Burn the x^T DMA
        # shadow with dense dummy matmuls so real work starts warm.
        warm_w = const.tile([128, 128], bf16, name="warm_w")
        warm_x = const.tile([128, QB], bf16, name="warm_x")
        nc.vector.memset(warm_w[:], 0.5)
        nc.vector.memset(warm_x[:], 0.5)
        # 12 warms ≈ the ~6.5us wk/wq/x-block-0 DMA window (cold 512ns each,
        # ~256ns once the ramp trips mid-burst) — sized so the PE never idles
        # a full MID window before the first projection.
        warm_ps = ps_s.tile([128, QB], fp32, tag="ps", name="warm_ps")
        for _ in range(12):
            nc.tensor.matmul(warm_ps[:], warm_w[:], warm_x[:], start=True, stop=True)

        # ---- persistent SBUF tensors ----
        qT = [const.tile([128, T], bf16, tag=f"qT{g}", name=f"qT{g}") for g in range(2)]
        kT = [const.tile([128, T], bf16, tag=f"kT{g}", name=f"kT{g}") for g in range(2)]
        vaug = const.tile([128, NKT, HG, 65], bf16, name="vaug")
        nc.vector.memset(vaug[:, :, :, 64:65], 1.0)

        def proj_qk(dst, wt, g, qb4, c0=0, c1=QB):
            ps = ps_s.tile([128, c1 - c0], fp32, tag="ps", name="ps")
            for c in range(4):
                nc.tensor.matmul(
                    ps[:],
                    wt[c][:, g * 128:(g + 1) * 128],
                    xTb[c][qb4][:, c0:c1],
                    start=(c == 0),
                    stop=(c == 3),
                )
            nc.vector.tensor_copy(
                dst[g][:, qb4 * QB + c0:qb4 * QB + c1], ps[:])

        def proj_v(tt):
            ps = ps_s.tile([128, OUTW], fp32, tag="ps", name="ps")
            for c in range(4):
                nc.tensor.matmul(
                    ps[:],
                    xTb[c][tt // 4][:, (tt % 4) * 128:(tt % 4 + 1) * 128],
                    wvT[c][:, 0:OUTW],
                    start=(c == 0),
                    stop=(c == 3),
                )
            nc.vector.tensor_copy(
                vaug[:, tt, :, 0:64],
                ps[:].rearrange("p (h d) -> p h d", h=HG),
            )

        def attn_unit(qb, g, fillers=None, prefill=None, last=False):
            """One (q-block, head-pair) attention unit.  The two heads'
            score matmuls per k-tile are row-tiled (partitions 0-63 /
            64-127) and run concurrently on the PE.  AV(kt-1) is emitted
            after scores(kt) (1-deep software pipeline) so the PE works
            through the exp latency.  fillers[j] = list of closures run
            as PE filler after batch j's scores."""
            nkt = qb * 4 + 4
            # prefill runs BEFORE the first scores: at a unit boundary the
            # first scores block on the st-slot WAR until ACT drains the
            # previous unit's diagonal-exp backlog, and the in-order PE
            # queue would stall fillers emitted behind them.
            if prefill:
                for f in prefill:
                    f()
            ot = ps_s.tile([128, 2, QB], fp32, tag="ot", bufs=1, name="ot")
            osb = osb_pool.tile([65, 2, QB], bf16, tag="osb", name="osb")
            pend = []  # (kt, pt, q0, width) awaiting AV matmuls (lag 2)

            def emit_av(kt, pt, q0, width):
                for i in range(2):
                    nc.tensor.matmul(
                        ot[0:65, i, q0:q0 + width],
                        vaug[:, kt, 2 * g + i, :],
                        pt[:, i, q0:q0 + width],
                        start=(kt == 0),
                        stop=(kt == nkt - 1),
                    )
                # O^T columns [0,256) are final after diagonal kt qb*4+1,
                # columns [256,512) after the last kt: cast each half as
                # soon as it is final (subtile deps) so most of the cast is
                # off the unit boundary and the ot PSUM slot frees early.
                if kt == qb * 4 + 1:
                    nc.vector.tensor_copy(osb[:, :, 0:256], ot[0:65, :, 0:256])
                    if last:
                        # ship the final unit's first half early so the
                        # kernel tail only transfers 256 columns per head
                        for i in range(2):
                            nc.sync.dma_start(
                                out=out_d[2 * g + i, :, qb, 0:256],
                                in_=osb[:, i, 0:256],
                            )
                elif last and kt == nkt - 2:
                    # taper the tail: cols [256,384) are final after kt14
                    nc.vector.tensor_copy(osb[:, :, 256:384],
                                          ot[0:65, :, 256:384])
                    for i in range(2):
                        nc.sync.dma_start(
                            out=out_d[2 * g + i, :, qb, 256:384],
                            in_=osb[:, i, 256:384],
                        )
                elif kt == nkt - 1:
                    lo = 384 if last else 256
                    nc.vector.tensor_copy(osb[:, :, lo:QB],
                                          ot[0:65, :, lo:QB])

            for kt in range(nkt):
                diag = kt >= qb * 4
                q0 = (kt - qb * 4) * 128 if diag else 0
                width = QB - q0
                st = ps_s.tile([128, 2, QB], fp32, tag="st", name="st")
                for i in range(2):
                    nc.tensor.matmul(
                        st[:, i, q0:q0 + width],
                        kT[g][64 * i:64 * i + 64, kt * 128:(kt + 1) * 128],
                        qT[g][64 * i:64 * i + 64,
                              qb * QB + q0:qb * QB + q0 + width],
                        start=True,
                        stop=True,
                    )
                if fillers:
                    for f in fillers.get(kt, ()):
                        f()
                # AV lags 2 batches so the unit's first AV (which waits on
                # the previous unit's ot WAR) issues behind two score pairs.
                if len(pend) == 2:
                    emit_av(*pend.pop(0))
                pt = pt_pool.tile([128, 2, QB], bf16, tag="pt", name="pt")
                nc.scalar.activation(
                    pt[:, :, q0:q0 + width], st[:, :, q0:q0 + width],
                    func=EXP, scale=0.125,
                )
                if diag:
                    for i in range(2):
                        nc.vector.tensor_mul(
                            pt[:, i, q0:q0 + 128], pt[:, i, q0:q0 + 128],
                            mask_sb[:],
                        )
                pend.append((kt, pt, q0, width))
            for p in pend:
                emit_av(*p)

            # stream out unnormalized O^T + denominator row (host divides
            # + transposes); the casts already happened in emit_av.  The
            # last unit's earlier columns went out early, leaving only the
            # final 128-col quarter after its last (256-col) exp.
            lo = 384 if last else 0
            nc.sync.dma_start(out=out_d[2 * g, :, qb, lo:QB],
                              in_=osb[:, 0, lo:QB])
            eng = nc.scalar if last else nc.sync
            eng.dma_start(out=out_d[2 * g + 1, :, qb, lo:QB],
                          in_=osb[:, 1, lo:QB])

        # ---- schedule ----
        # qb ASCENDING: unit (0, g0) needs only x block 0 (K chunk 0, Q
        # chunk 0, V tiles 0-3), so the exp stream starts as soon as the
        # first quarter of x^T lands.  Each unit's fillers project what the
        # NEXT units need, in x-arrival order.  An 8-warm bridge covers the
        # wq DMA wait.
        # NOTE: a narrow (N=128) first K projection starts the exp stream
        # earlier on paper but thins the PE right at the HAM window edge —
        # measured: MID re-throttle at ~16us and a HALF-CLOCK first unit.
        # Keep the prologue dense.
        proj_qk(kT, wkT, 0, 0)
        for _ in range(2):
            nc.tensor.matmul(warm_ps[:], warm_w[:], warm_x[:], start=True, stop=True)
        proj_qk(qT, wqT, 0, 0)

        def F(*items):
            out = []
            for it in items:
                if it[0] == "v":
                    out.append(lambda t=it[1]: proj_v(t))
                elif it[0] == "k":
                    out.append(lambda g=it[1], c=it[2]: proj_qk(kT, wkT, g, c))
                else:
                    out.append(lambda g=it[1], c=it[2]: proj_qk(qT, wqT, g, c))
            return out

        fill_00 = {
            0: F(("v", 0)),
            1: F(("v", 1)),
            2: F(("v", 2), ("k", 1, 0)),
            3: F(("v", 3), ("q", 1, 0)),
        }
        fill_01 = {
            1: F(("k", 0, 1)),
            2: F(("q", 0, 1)),
            3: F(("v", 4)),
        }
        fill_10 = {
            0: F(("v", 5)),
            1: F(("k", 1, 1)),
            2: F(("q", 1, 1)),
            3: F(("v", 6)),
            4: F(("v", 7)),
            5: F(("k", 0, 2)),
            6: F(("v", 8)),
            7: F(("v", 9)),
        }
        fill_11 = {
            0: F(("q", 0, 2)),
            1: F(("v", 10)),
            2: F(("v", 11)),
            4: F(("q", 1, 2)),
            5: F(("k", 0, 3)),
            6: F(("v", 12)),
            7: F(("v", 13)),
        }
        fill_20 = {
            0: F(("v", 14)),
            1: F(("v", 15)),
            2: F(("q", 0, 3)),
        }
        attn_unit(0, 0, fillers=fill_00)
        attn_unit(0, 1, fillers=fill_01)
        attn_unit(1, 0, fillers=fill_10)
        attn_unit(1, 1, fillers=fill_11)
        attn_unit(2, 0, fillers=fill_20)
        # the last projections are saved as PREFILL for the otherwise
        # fillerless late units (see attn_unit)
        attn_unit(2, 1, prefill=F(("k", 1, 2)))
        attn_unit(3, 0, prefill=F(("k", 1, 3)))
        attn_unit(3, 1, prefill=F(("q", 1, 3)), last=True)

    nc.finalize()
    return nc


def _get_nc():
    if "nc" not in _CACHE:
        _CACHE["nc"] = _build_nc()
    return _CACHE["nc"]


def _make_cmask():
    # triangle: mask[p, f] = 1.0 iff p <= f
    p = np.arange(128)[:, None]
    f = np.arange(128)[None, :]
    return (p <= f).astype(ml_dtypes.bfloat16)


def _make_in_maps(x, Wq, Wk, Wv):
    bf = ml_dtypes.bfloat16
    cmask = _make_cmask()
    in_maps = []
    for c in range(N_CORES):
        b, hg = c // 2, c % 2
        r0 = hg * OUTW
        in_maps.append({
            "xt": np.ascontiguousarray(x[b].T).astype(bf),
            "wqt": np.ascontiguousarray(Wq[r0:r0 + OUTW].T).astype(bf),
            "wkt": np.ascontiguousarray(Wk[r0:r0 + OUTW].T).astype(bf),
            "wvt": np.ascontiguousarray(Wv[r0:r0 + OUTW].T).astype(bf),
            "cmask": cmask,
        })
    return in_maps


def _postprocess(results, B):
    """Host-side unshard: divide unnormalized O^T by the denominator row
    and transpose to natural [T, D] layout."""
    out = np.empty((B, T, D), dtype=np.float32)
    for c in range(N_CORES):
        b, hg = c // 2, c % 2
        r = results[c]["out"].astype(np.float32)  # [HG, 65, NQB, QB]
        r = r.reshape(HG, 65, T)
        for h in range(HG):
            blk = r[h, :64] / r[h, 64:65]  # [64, T]
            out[b, :, hg * OUTW + h * DH:hg * OUTW + (h + 1) * DH] = blk.T
    return out


def kernel(x, Wq, Wk, Wv):
    from concourse.bass_utils import run_bass_kernel_spmd

    nc = _get_nc()
    in_maps = _make_in_maps(x, Wq, Wk, Wv)
    res = run_bass_kernel_spmd(nc, in_maps, core_ids=list(range(N_CORES)))
    return _postprocess(res.results, x.shape[0])


# revision 41
# speedup vs baseline: 1.1845x; 1.0205x over previous
"""Multi-head causal attention (B=4, T=2048, D=512, H=8) on 8 TRN2 NeuronCores.

Sharding: core c handles batch b = c//2 and head-group hg = c%2 (4 heads,
256 output dims).  No collectives needed — 8 fully independent problems.

Per-core algorithm (matmul inputs bf16, O^T accumulation f32 in PSUM):
  - host passes x^T (D,T) and W^T slices (D, 256) in bf16 + a [128,128]
    triangular causal mask
  - Q^T,K^T projections:  qT[dh2,T] = W2h @ xT, two heads stacked per tile
    (head 2g at partitions 0-63, head 2g+1 at partitions 64-127)
  - V projection into augmented-V tiles [k-tile 128, 65] (ones column
    appended -> the O^T matmul also produces the softmax denominator row).
    The ones-column is OPTIMAL for the denominator: any scheme that reads
    the 8.9M-element P matrix on another engine (DVE/gpsimd adds for
    col-packed M=64+64 AV) costs >= the AV-packing saving.
  - flash-style over head-PAIRS: for each (q-block, pair g), per k-tile the
    two heads' score matmuls S^T[k,q] = K^T.T @ Q^T are row-tiled
    (tile_position (0,0)/(64,0) via base_partition auto-derive) and stream
    CONCURRENTLY on the PE (measured dstart 3ns) — K=64 each, so the pair
    costs one matmul's stream time.  exp via one ACT instr per k-tile over
    both heads [128, 2, width] (scale=1/8 folded; no max subtraction:
    |scores| < ~4); causal via per-k-tile width restriction + triangle-mask
    multiply on the boundary block (the in-block triangle waste lives in
    the partition dim and is free on every engine).
  - O^T accumulated in PSUM over k-tiles (start/stop groups), software
    pipelined TWO batches deep (AV(kt) emitted after scores(kt+2)) so the
    PE rides through exp latency AND through the ot-slot WAR at unit entry.
  - epilogue per unit: O^T+denominator cast to bf16 in column HALVES —
    cols [0,256) are final right after diagonal kt qb*4+1, so most of the
    cast happens mid-unit and the single ot PSUM slot frees early — then
    DMA'd out UNNORMALIZED; the host divides by the denominator row and
    transposes (removes all PE transposes + DVE normalize work).

Scheduling (program order == Tile priority): 12 warm matmuls burn the
x-block-0 DMA shadow (HAM ramp) + 2-warm bridge over the wq wait; then
units run qb ASCENDING (unit (0,g0) needs only x block 0 for K chunk 0,
Q chunk 0 and V tiles 0-3, so the exp stream starts as soon as the first
quarter of x^T lands), g=0 then 1, with later projections woven between
batches as PE filler in x-arrival order.  The last three units have no
fillers left; their first projections are PREFILLED before the unit's
first scores (in-order PE queue: a filler emitted behind st-WAR-blocked
scores is itself stalled).  x^T is DMA'd as 16 column-block pieces over
the SP/ACT/gpsimd DGE queues, ACT-queue pieces all landing before the
first exp; outputs stream per unit on SP (last unit split SP/ACT).

PSUM budget (8 banks x 2KB): st [128,2,512] f32 x2 bufs (4) + ot
[128,2,512] x1 (2) + proj ps [128,512] x2 (2).

Measured on this container's device (PE ~2.45GHz warm): 110.2-110.9us
over 10 reps, rel_err 3.8e-3.  The device drifts between ~2.45GHz and a
~2.0GHz P0 power state run-to-run (~+18% exec when hot) — compare runs
via the score-MM median duration (372ns warm vs 446ns hot), not raw ns.
Fixed overheads outside kernel control: ~7.2us preamble (barrier rings +
ACT table load before any DMA trigger), ~6.5us teardown semaphore chain.
"""

import numpy as np
import ml_dtypes

T = 2048
D = 512
HG = 4  # heads per core
DH = 64
OUTW = HG * DH  # 256
QB = 512  # q block (columns of S^T tiles)
NQB = T // QB  # 4
NKT = T // 128  # 16 k-tiles
N_CORES = 8

_CACHE = {}


def _build_nc():
    import concourse.bacc as bacc
    import concourse.tile as tile
    import concourse.mybir as mybir
    from contextlib import ExitStack

    fp32 = mybir.dt.float32
    bf16 = mybir.dt.bfloat16
    EXP = mybir.ActivationFunctionType.Exp

    nc = bacc.Bacc(None, target_bir_lowering=False)

    xt_d = nc.declare_dram_parameter("xt", [D, T], bf16, isOutput=False)
    wqt_d = nc.declare_dram_parameter("wqt", [D, OUTW], bf16, isOutput=False)
    wkt_d = nc.declare_dram_parameter("wkt", [D, OUTW], bf16, isOutput=False)
    wvt_d = nc.declare_dram_parameter("wvt", [D, OUTW], bf16, isOutput=False)
    cmask_d = nc.declare_dram_parameter("cmask", [128, 128], bf16, isOutput=False)
    # unnormalized O^T + denominator row: [head, 65, qb, 512]
    out_d = nc.declare_dram_parameter("out", [HG, 65, NQB, QB], bf16, isOutput=True)

    with tile.TileContext(nc) as tc, ExitStack() as ctx:
        const = ctx.enter_context(tc.tile_pool(name="const", bufs=1))
        ps_s = ctx.enter_context(tc.tile_pool(name="ps_s", bufs=2, space="PSUM"))
        pt_pool = ctx.enter_context(tc.tile_pool(name="pt", bufs=4))
        osb_pool = ctx.enter_context(tc.tile_pool(name="osb", bufs=2))

        # ---- input loads ----
        # x^T arrives as 16 column-block pieces (chunk c x q-block b), DMA'd
        # in ascending consumption order (block 0 gates the prologue
        # projections) and spread over the three DGE queues.  The scalar
        # (ACT) queue only carries pieces that finish BEFORE the first exp,
        # so triggers never steal ACT time from the exp stream.
        xTb = [[const.tile([128, QB], bf16, tag=f"xT{c}_{b}", name=f"xT{c}_{b}")
                for b in range(4)] for c in range(4)]
        wkT = [const.tile([128, OUTW], bf16, tag=f"wkT{c}", name=f"wkT{c}")
               for c in range(4)]
        wqT = [const.tile([128, OUTW], bf16, tag=f"wqT{c}", name=f"wqT{c}")
               for c in range(4)]
        wvT = [const.tile([128, OUTW], bf16, tag=f"wvT{c}", name=f"wvT{c}")
               for c in range(4)]
        mask_sb = const.tile([128, 128], bf16, name="mask_sb")

        def ld_w(eng, wt, dram, c):
            eng.dma_start(out=wt[c][:], in_=dram[c * 128:(c + 1) * 128, :])

        def ld_x(eng, c, b):
            eng.dma_start(
                out=xTb[c][b][:],
                in_=xt_d[c * 128:(c + 1) * 128, b * QB:(b + 1) * QB],
            )

        ld_w(nc.sync, wkT, wkt_d, 0)
        ld_w(nc.sync, wkT, wkt_d, 1)
        ld_w(nc.sync, wkT, wkt_d, 3)
        ld_x(nc.sync, 0, 0)
        ld_w(nc.sync, wqT, wqt_d, 0)
        ld_w(nc.sync, wqT, wqt_d, 1)
        ld_x(nc.sync, 0, 1)
        ld_x(nc.sync, 1, 1)
        ld_x(nc.sync, 2, 1)
        ld_x(nc.sync, 3, 1)
        ld_x(nc.sync, 0, 3)
        ld_x(nc.sync, 1, 3)

        ld_w(nc.scalar, wkT, wkt_d, 2)
        ld_x(nc.scalar, 2, 0)
        ld_x(nc.scalar, 1, 0)
        ld_w(nc.scalar, wqT, wqt_d, 2)
        ld_w(nc.scalar, wqT, wqt_d, 3)
        nc.scalar.dma_start(out=mask_sb[:], in_=cmask_d[:])

        # slow SWDGE triggers (~1us each) only for the one block-0 piece
        # that balances the fast queues, plus late-needed pieces
        ld_x(nc.gpsimd, 3, 0)
        for c in range(4):
            ld_w(nc.gpsimd, wvT, wvt_d, c)
        ld_x(nc.gpsimd, 0, 2)
        ld_x(nc.gpsimd, 1, 2)
        ld_x(nc.gpsimd, 2, 2)
        ld_x(nc.gpsimd, 3, 2)
        ld_x(nc.gpsimd, 2, 3)
        ld_x(nc.gpsimd, 3, 3)

        # ---- HAM warm-up burst ----
        # The PE clock needs a fully-busy window to ramp.  Burn the x^T DMA
        # shadow with dense dummy matmuls so real work starts warm.
        warm_w = const.tile([128, 128], bf16, name="warm_w")
        warm_x = const.tile([128, QB], bf16, name="warm_x")
        nc.vector.memset(warm_w[:], 0.5)
        nc.vector.memset(warm_x[:], 0.5)
        # 12 warms ≈ the ~6.5us wk/wq/x-block-0 DMA window (cold 512ns each,
        # ~256ns once the ramp trips mid-burst) — sized so the PE never idles
        # a full MID window before the first projection.
        warm_ps = ps_s.tile([128, QB], fp32, tag="ps", name="warm_ps")
        for _ in range(12):
            nc.tensor.matmul(warm_ps[:], warm_w[:], warm_x[:], start=True, stop=True)

        # ---- persistent SBUF tensors ----
        qT = [const.tile([128, T], bf16, tag=f"qT{g}", name=f"qT{g}") for g in range(2)]
        kT = [const.tile([128, T], bf16, tag=f"kT{g}", name=f"kT{g}") for g in range(2)]
        vaug = const.tile([128, NKT, HG, 65], bf16, name="vaug")
        nc.vector.memset(vaug[:, :, :, 64:65], 1.0)

        def proj_qk(dst, wt, g, qb4, c0=0, c1=QB):
            ps = ps_s.tile([128, c1 - c0], fp32, tag="ps", name="ps")
            for c in range(4):
                nc.tensor.matmul(
                    ps[:],
                    wt[c][:, g * 128:(g + 1) * 128],
                    xTb[c][qb4][:, c0:c1],
                    start=(c == 0),
                    stop=(c == 3),
                )
            nc.vector.tensor_copy(
                dst[g][:, qb4 * QB + c0:qb4 * QB + c1], ps[:])

        def proj_v(tt):
            ps = ps_s.tile([128, OUTW], fp32, tag="ps", name="ps")
            for c in range(4):
                nc.tensor.matmul(
                    ps[:],
                    xTb[c][tt // 4][:, (tt % 4) * 128:(tt % 4 + 1) * 128],
                    wvT[c][:, 0:OUTW],
                    start=(c == 0),
                    stop=(c == 3),
                )
            nc.vector.tensor_copy(
                vaug[:, tt, :, 0:64],
                ps[:].rearrange("p (h d) -> p h d", h=HG),
            )

        def attn_unit(qb, g, fillers=None, prefill=None, last=False):
            """One (q-block, head-pair) attention unit.  The two heads'
            score matmuls per k-tile are row-tiled (partitions 0-63 /
            64-127) and run concurrently on the PE.  AV(kt-1) is emitted
            after scores(kt) (1-deep software pipeline) so the PE works
            through the exp latency.  fillers[j] = list of closures run
            as PE filler after batch j's scores."""
            nkt = qb * 4 + 4
            # prefill runs BEFORE the first scores: at a unit boundary the
            # first scores block on the st-slot WAR until ACT drains the
            # previous unit's diagonal-exp backlog, and the in-order PE
            # queue would stall fillers emitted behind them.
            if prefill:
                for f in prefill:
                    f()
            ot = ps_s.tile([128, 2, QB], fp32, tag="ot", bufs=1, name="ot")
            osb = osb_pool.tile([65, 2, QB], bf16, tag="osb", name="osb")
            pend = []  # (kt, pt, q0, width) awaiting AV matmuls (lag 2)

            def emit_av(kt, pt, q0, width):
                for i in range(2):
                    nc.tensor.matmul(
                        ot[0:65, i, q0:q0 + width],
                        vaug[:, kt, 2 * g + i, :],
                        pt[:, i, q0:q0 + width],
                        start=(kt == 0),
                        stop=(kt == nkt - 1),
                    )
                # O^T columns [0,256) are final after diagonal kt qb*4+1,
                # columns [256,512) after the last kt: cast each half as
                # soon as it is final (subtile deps) so most of the cast is
                # off the unit boundary and the ot PSUM slot frees early.
                if kt == qb * 4 + 1:
                    nc.vector.tensor_copy(osb[:, :, 0:256], ot[0:65, :, 0:256])
                    if last:
                        # ship the final unit's first half early so the
                        # kernel tail only transfers 256 columns per head
                        for i in range(2):
                            nc.sync.dma_start(
                                out=out_d[2 * g + i, :, qb, 0:256],
                                in_=osb[:, i, 0:256],
                            )
                elif kt == nkt - 1:
                    nc.vector.tensor_copy(osb[:, :, 256:QB],
                                          ot[0:65, :, 256:QB])

            for kt in range(nkt):
                diag = kt >= qb * 4
                q0 = (kt - qb * 4) * 128 if diag else 0
                width = QB - q0
                st = ps_s.tile([128, 2, QB], fp32, tag="st", name="st")
                for i in range(2):
                    nc.tensor.matmul(
                        st[:, i, q0:q0 + width],
                        kT[g][64 * i:64 * i + 64, kt * 128:(kt + 1) * 128],
                        qT[g][64 * i:64 * i + 64,
                              qb * QB + q0:qb * QB + q0 + width],
                        start=True,
                        stop=True,
                    )
                if fillers:
                    for f in fillers.get(kt, ()):
                        f()
                # AV lags 2 batches so the unit's first AV (which waits on
                # the previous unit's ot WAR) issues behind two score pairs.
                if len(pend) == 2:
                    emit_av(*pend.pop(0))
                pt = pt_pool.tile([128, 2, QB], bf16, tag="pt", name="pt")
                nc.scalar.activation(
                    pt[:, :, q0:q0 + width], st[:, :, q0:q0 + width],
                    func=EXP, scale=0.125,
                )
                if diag:
                    for i in range(2):
                        nc.vector.tensor_mul(
                            pt[:, i, q0:q0 + 128], pt[:, i, q0:q0 + 128],
                            mask_sb[:],
                        )
                pend.append((kt, pt, q0, width))
            for p in pend:
                emit_av(*p)

            # stream out unnormalized O^T + denominator row (host divides
            # + transposes); the casts already happened in emit_av.
            # (Tapering further into per-quarter DMAs was measured WORSE:
            # the extra sync triggers land in the ACT-paced final stretch.)
            lo = 256 if last else 0  # last unit's first half went out early
            nc.sync.dma_start(out=out_d[2 * g, :, qb, lo:QB],
                              in_=osb[:, 0, lo:QB])
            eng = nc.scalar if last else nc.sync
            eng.dma_start(out=out_d[2 * g + 1, :, qb, lo:QB],
                          in_=osb[:, 1, lo:QB])

        # ---- schedule ----
        # qb ASCENDING: unit (0, g0) needs only x block 0 (K chunk 0, Q
        # chunk 0, V tiles 0-3), so the exp stream starts as soon as the
        # first quarter of x^T lands.  Each unit's fillers project what the
        # NEXT units need, in x-arrival order.  An 8-warm bridge covers the
        # wq DMA wait.
        # NOTE: a narrow (N=128) first K projection starts the exp stream
        # earlier on paper but thins the PE right at the HAM window edge —
        # measured: MID re-throttle at ~16us and a HALF-CLOCK first unit.
        # Keep the prologue dense.
        proj_qk(kT, wkT, 0, 0)
        for _ in range(2):
            nc.tensor.matmul(warm_ps[:], warm_w[:], warm_x[:], start=True, stop=True)
        proj_qk(qT, wqT, 0, 0)

        def F(*items):
            out = []
            for it in items:
                if it[0] == "v":
                    out.append(lambda t=it[1]: proj_v(t))
                elif it[0] == "k":
                    out.append(lambda g=it[1], c=it[2]: proj_qk(kT, wkT, g, c))
                else:
                    out.append(lambda g=it[1], c=it[2]: proj_qk(qT, wqT, g, c))
            return out

        fill_00 = {
            0: F(("v", 0)),
            1: F(("v", 1)),
            2: F(("v", 2), ("k", 1, 0)),
            3: F(("v", 3), ("q", 1, 0)),
        }
        fill_01 = {
            1: F(("k", 0, 1)),
            2: F(("q", 0, 1)),
            3: F(("v", 4)),
        }
        fill_10 = {
            0: F(("v", 5)),
            1: F(("k", 1, 1)),
            2: F(("q", 1, 1)),
            3: F(("v", 6)),
            4: F(("v", 7)),
            5: F(("k", 0, 2)),
            6: F(("v", 8)),
            7: F(("v", 9)),
        }
        fill_11 = {
            0: F(("q", 0, 2)),
            1: F(("v", 10)),
            2: F(("v", 11)),
            4: F(("q", 1, 2)),
            5: F(("k", 0, 3)),
            6: F(("v", 12)),
            7: F(("v", 13)),
        }
        fill_20 = {
            0: F(("v", 14)),
            1: F(("v", 15)),
            2: F(("q", 0, 3)),
        }
        attn_unit(0, 0, fillers=fill_00)
        attn_unit(0, 1, fillers=fill_01)
        attn_unit(1, 0, fillers=fill_10)
        attn_unit(1, 1, fillers=fill_11)
        attn_unit(2, 0, fillers=fill_20)
        # the last projections are saved as PREFILL for the otherwise
        # fillerless late units (see attn_unit)
        attn_unit(2, 1, prefill=F(("k", 1, 2)))
        attn_unit(3, 0, prefill=F(("k", 1, 3)))
        attn_unit(3, 1, prefill=F(("q", 1, 3)), last=True)

    nc.finalize()
    return nc


def _get_nc():
    if "nc" not in _CACHE:
        _CACHE["nc"] = _build_nc()
    return _CACHE["nc"]


def _make_cmask():
    # triangle: mask[p, f] = 1.0 iff p <= f
    p = np.arange(128)[:, None]
    f = np.arange(128)[None, :]
    return (p <= f).astype(ml_dtypes.bfloat16)


def _make_in_maps(x, Wq, Wk, Wv):
    bf = ml_dtypes.bfloat16
    cmask = _make_cmask()
    in_maps = []
    for c in range(N_CORES):
        b, hg = c // 2, c % 2
        r0 = hg * OUTW
        in_maps.append({
            "xt": np.ascontiguousarray(x[b].T).astype(bf),
            "wqt": np.ascontiguousarray(Wq[r0:r0 + OUTW].T).astype(bf),
            "wkt": np.ascontiguousarray(Wk[r0:r0 + OUTW].T).astype(bf),
            "wvt": np.ascontiguousarray(Wv[r0:r0 + OUTW].T).astype(bf),
            "cmask": cmask,
        })
    return in_maps


def _postprocess(results, B):
    """Host-side unshard: divide unnormalized O^T by the denominator row
    and transpose to natural [T, D] layout."""
    out = np.empty((B, T, D), dtype=np.float32)
    for c in range(N_CORES):
        b, hg = c // 2, c % 2
        r = results[c]["out"].astype(np.float32)  # [HG, 65, NQB, QB]
        r = r.reshape(HG, 65, T)
        for h in range(HG):
            blk = r[h, :64] / r[h, 64:65]  # [64, T]
            out[b, :, hg * OUTW + h * DH:hg * OUTW + (h + 1) * DH] = blk.T
    return out


def kernel(x, Wq, Wk, Wv):
    from concourse.bass_utils import run_bass_kernel_spmd

    nc = _get_nc()
    in_maps = _make_in_maps(x, Wq, Wk, Wv)
    res = run_bass_kernel_spmd(nc, in_maps, core_ids=list(range(N_CORES)))
    return _postprocess(res.results, x.shape[0])


# revision 42
# speedup vs baseline: 1.1846x; 1.0001x over previous
"""Multi-head causal attention (B=4, T=2048, D=512, H=8) on 8 TRN2 NeuronCores.

Sharding: core c handles batch b = c//2 and head-group hg = c%2 (4 heads,
256 output dims).  No collectives needed — 8 fully independent problems.

Per-core algorithm (matmul inputs bf16, O^T accumulation f32 in PSUM):
  - host passes x^T (D,T) and W^T slices (D, 256) in bf16 + a [128,128]
    triangular causal mask
  - Q^T,K^T projections:  qT[dh2,T] = W2h @ xT, two heads stacked per tile
    (head 2g at partitions 0-63, head 2g+1 at partitions 64-127)
  - V projection into augmented-V tiles [k-tile 128, 65] (ones column
    appended -> the O^T matmul also produces the softmax denominator row).
    The ones-column is OPTIMAL for the denominator: any scheme that reads
    the 8.9M-element P matrix on another engine (DVE/gpsimd adds for
    col-packed M=64+64 AV) costs >= the AV-packing saving.
  - flash-style over head-PAIRS: for each (q-block, pair g), per k-tile the
    two heads' score matmuls S^T[k,q] = K^T.T @ Q^T are row-tiled
    (tile_position (0,0)/(64,0) via base_partition auto-derive) and stream
    CONCURRENTLY on the PE (measured dstart 3ns) — K=64 each, so the pair
    costs one matmul's stream time.  exp via one ACT instr per k-tile over
    both heads [128, 2, width] (scale=1/8 folded; no max subtraction:
    |scores| < ~4); causal via per-k-tile width restriction + triangle-mask
    multiply on the boundary block (the in-block triangle waste lives in
    the partition dim and is free on every engine).
  - O^T accumulated in PSUM over k-tiles (start/stop groups), software
    pipelined TWO batches deep (AV(kt) emitted after scores(kt+2)) so the
    PE rides through exp latency AND through the ot-slot WAR at unit entry.
  - epilogue per unit: O^T+denominator cast to bf16 in column HALVES —
    cols [0,256) are final right after diagonal kt qb*4+1, so most of the
    cast happens mid-unit and the single ot PSUM slot frees early — then
    DMA'd out UNNORMALIZED; the host divides by the denominator row and
    transposes (removes all PE transposes + DVE normalize work).

Scheduling (program order == Tile priority): 12 warm matmuls burn the
x-block-0 DMA shadow (HAM ramp) + 2-warm bridge over the wq wait; then
units run qb ASCENDING (unit (0,g0) needs only x block 0 for K chunk 0,
Q chunk 0 and V tiles 0-3, so the exp stream starts as soon as the first
quarter of x^T lands), g=0 then 1, with later projections woven between
batches as PE filler in x-arrival order.  The last three units have no
fillers left; their first projections are PREFILLED before the unit's
first scores (in-order PE queue: a filler emitted behind st-WAR-blocked
scores is itself stalled).  x^T is DMA'd as 16 column-block pieces over
the SP/ACT/gpsimd DGE queues, ACT-queue pieces all landing before the
first exp; outputs stream per unit on SP (last unit split SP/ACT).

PSUM budget (8 banks x 2KB): st [128,2,512] f32 x2 bufs (4) + ot
[128,2,512] x1 (2) + proj ps [128,512] x2 (2).

Measured on this container's device (PE ~2.45GHz warm): 110.2-110.9us
over 10 reps, rel_err 3.8e-3.  The device drifts between ~2.45GHz and a
~2.0GHz P0 power state run-to-run (~+18% exec when hot) — compare runs
via the score-MM median duration (372ns warm vs 446ns hot), not raw ns.
Fixed overheads outside kernel control: ~7.2us preamble (barrier rings +
ACT table load before any DMA trigger), ~6.5us teardown semaphore chain.
"""

import numpy as np
import ml_dtypes

T = 2048
D = 512
HG = 4  # heads per core
DH = 64
OUTW = HG * DH  # 256
QB = 512  # q block (columns of S^T tiles)
NQB = T // QB  # 4
NKT = T // 128  # 16 k-tiles
N_CORES = 8

_CACHE = {}


def _build_nc():
    import concourse.bacc as bacc
    import concourse.tile as tile
    import concourse.mybir as mybir
    from contextlib import ExitStack

    fp32 = mybir.dt.float32
    bf16 = mybir.dt.bfloat16
    EXP = mybir.ActivationFunctionType.Exp

    nc = bacc.Bacc(None, target_bir_lowering=False)

    xt_d = nc.declare_dram_parameter("xt", [D, T], bf16, isOutput=False)
    wqt_d = nc.declare_dram_parameter("wqt", [D, OUTW], bf16, isOutput=False)
    wkt_d = nc.declare_dram_parameter("wkt", [D, OUTW], bf16, isOutput=False)
    wvt_d = nc.declare_dram_parameter("wvt", [D, OUTW], bf16, isOutput=False)
    cmask_d = nc.declare_dram_parameter("cmask", [128, 128], bf16, isOutput=False)
    # unnormalized O^T + denominator row: [head, 65, qb, 512]
    out_d = nc.declare_dram_parameter("out", [HG, 65, NQB, QB], bf16, isOutput=True)

    with tile.TileContext(nc) as tc, ExitStack() as ctx:
        const = ctx.enter_context(tc.tile_pool(name="const", bufs=1))
        ps_s = ctx.enter_context(tc.tile_pool(name="ps_s", bufs=2, space="PSUM"))
        pt_pool = ctx.enter_context(tc.tile_pool(name="pt", bufs=4))
        osb_pool = ctx.enter_context(tc.tile_pool(name="osb", bufs=2))

        # ---- input loads ----
        # x^T arrives as 16 column-block pieces (chunk c x q-block b), DMA'd
        # in ascending consumption order (block 0 gates the prologue
        # projections) and spread over the three DGE queues.  The scalar
        # (ACT) queue only carries pieces that finish BEFORE the first exp,
        # so triggers never steal ACT time from the exp stream.
        xTb = [[const.tile([128, QB], bf16, tag=f"xT{c}_{b}", name=f"xT{c}_{b}")
                for b in range(4)] for c in range(4)]
        wkT = [const.tile([128, OUTW], bf16, tag=f"wkT{c}", name=f"wkT{c}")
               for c in range(4)]
        wqT = [const.tile([128, OUTW], bf16, tag=f"wqT{c}", name=f"wqT{c}")
               for c in range(4)]
        wvT = [const.tile([128, OUTW], bf16, tag=f"wvT{c}", name=f"wvT{c}")
               for c in range(4)]
        mask_sb = const.tile([128, 128], bf16, name="mask_sb")

        def ld_w(eng, wt, dram, c):
            eng.dma_start(out=wt[c][:], in_=dram[c * 128:(c + 1) * 128, :])

        def ld_x(eng, c, b):
            eng.dma_start(
                out=xTb[c][b][:],
                in_=xt_d[c * 128:(c + 1) * 128, b * QB:(b + 1) * QB],
            )

        ld_w(nc.sync, wkT, wkt_d, 0)
        ld_w(nc.sync, wkT, wkt_d, 1)
        ld_w(nc.sync, wkT, wkt_d, 3)
        ld_x(nc.sync, 0, 0)
        ld_w(nc.sync, wqT, wqt_d, 0)
        ld_w(nc.sync, wqT, wqt_d, 1)
        ld_x(nc.sync, 0, 1)
        ld_x(nc.sync, 1, 1)
        ld_x(nc.sync, 2, 1)
        ld_x(nc.sync, 3, 1)
        ld_x(nc.sync, 0, 3)
        ld_x(nc.sync, 1, 3)

        ld_w(nc.scalar, wkT, wkt_d, 2)
        ld_x(nc.scalar, 2, 0)
        ld_x(nc.scalar, 1, 0)
        ld_w(nc.scalar, wqT, wqt_d, 2)
        ld_w(nc.scalar, wqT, wqt_d, 3)
        nc.scalar.dma_start(out=mask_sb[:], in_=cmask_d[:])

        # slow SWDGE triggers (~1us each) only for the one block-0 piece
        # that balances the fast queues, plus late-needed pieces
        ld_x(nc.gpsimd, 3, 0)
        for c in range(4):
            ld_w(nc.gpsimd, wvT, wvt_d, c)
        ld_x(nc.gpsimd, 0, 2)
        ld_x(nc.gpsimd, 1, 2)
        ld_x(nc.gpsimd, 2, 2)
        ld_x(nc.gpsimd, 3, 2)
        ld_x(nc.gpsimd, 2, 3)
        ld_x(nc.gpsimd, 3, 3)

        # ---- HAM warm-up burst ----
        # The PE clock needs a fully-busy window to ramp.  Burn the x^T DMA
        # shadow with dense dummy matmuls so real work starts warm.
        warm_w = const.tile([128, 128], bf16, name="warm_w")
        warm_x = const.tile([128, QB], bf16, name="warm_x")
        nc.vector.memset(warm_w[:], 0.5)
        nc.vector.memset(warm_x[:], 0.5)
        # 12 warms ≈ the ~6.5us wk/wq/x-block-0 DMA window (cold 512ns each,
        # ~256ns once the ramp trips mid-burst) — sized so the PE never idles
        # a full MID window before the first projection.
        warm_ps = ps_s.tile([128, QB], fp32, tag="ps", name="warm_ps")
        for _ in range(12):
            nc.tensor.matmul(warm_ps[:], warm_w[:], warm_x[:], start=True, stop=True)

        # ---- persistent SBUF tensors ----
        qT = [const.tile([128, T], bf16, tag=f"qT{g}", name=f"qT{g}") for g in range(2)]
        kT = [const.tile([128, T], bf16, tag=f"kT{g}", name=f"kT{g}") for g in range(2)]
        vaug = const.tile([128, NKT, HG, 65], bf16, name="vaug")
        nc.vector.memset(vaug[:, :, :, 64:65], 1.0)

        def proj_qk(dst, wt, g, qb4, c0=0, c1=QB):
            ps = ps_s.tile([128, c1 - c0], fp32, tag="ps", name="ps")
            for c in range(4):
                nc.tensor.matmul(
                    ps[:],
                    wt[c][:, g * 128:(g + 1) * 128],
                    xTb[c][qb4][:, c0:c1],
                    start=(c == 0),
                    stop=(c == 3),
                )
            nc.vector.tensor_copy(
                dst[g][:, qb4 * QB + c0:qb4 * QB + c1], ps[:])

        def proj_v(tt):
            ps = ps_s.tile([128, OUTW], fp32, tag="ps", name="ps")
            for c in range(4):
                nc.tensor.matmul(
                    ps[:],
                    xTb[c][tt // 4][:, (tt % 4) * 128:(tt % 4 + 1) * 128],
                    wvT[c][:, 0:OUTW],
                    start=(c == 0),
                    stop=(c == 3),
                )
            nc.vector.tensor_copy(
                vaug[:, tt, :, 0:64],
                ps[:].rearrange("p (h d) -> p h d", h=HG),
            )

        def attn_unit(qb, g, fillers=None, prefill=None, last=False):
            """One (q-block, head-pair) attention unit.  The two heads'
            score matmuls per k-tile are row-tiled (partitions 0-63 /
            64-127) and run concurrently on the PE.  AV(kt-1) is emitted
            after scores(kt) (1-deep software pipeline) so the PE works
            through the exp latency.  fillers[j] = list of closures run
            as PE filler after batch j's scores."""
            nkt = qb * 4 + 4
            # prefill runs BEFORE the first scores: at a unit boundary the
            # first scores block on the st-slot WAR until ACT drains the
            # previous unit's diagonal-exp backlog, and the in-order PE
            # queue would stall fillers emitted behind them.
            if prefill:
                for f in prefill:
                    f()
            ot = ps_s.tile([128, 2, QB], fp32, tag="ot", bufs=1, name="ot")
            osb = osb_pool.tile([65, 2, QB], bf16, tag="osb", name="osb")
            pend = []  # (kt, pt, q0, width) awaiting AV matmuls (lag 2)

            def emit_av(kt, pt, q0, width):
                for i in range(2):
                    nc.tensor.matmul(
                        ot[0:65, i, q0:q0 + width],
                        vaug[:, kt, 2 * g + i, :],
                        pt[:, i, q0:q0 + width],
                        start=(kt == 0),
                        stop=(kt == nkt - 1),
                    )
                # O^T columns [0,256) are final after diagonal kt qb*4+1,
                # columns [256,512) after the last kt: cast each half as
                # soon as it is final (subtile deps) so most of the cast is
                # off the unit boundary and the ot PSUM slot frees early.
                if kt == qb * 4 + 1:
                    nc.vector.tensor_copy(osb[:, :, 0:256], ot[0:65, :, 0:256])
                    if last:
                        # ship the final unit's first half early so the
                        # kernel tail only transfers 256 columns per head
                        for i in range(2):
                            nc.sync.dma_start(
                                out=out_d[2 * g + i, :, qb, 0:256],
                                in_=osb[:, i, 0:256],
                            )
                elif kt == nkt - 1:
                    nc.vector.tensor_copy(osb[:, :, 256:QB],
                                          ot[0:65, :, 256:QB])

            for kt in range(nkt):
                diag = kt >= qb * 4
                q0 = (kt - qb * 4) * 128 if diag else 0
                width = QB - q0
                st = ps_s.tile([128, 2, QB], fp32, tag="st", name="st")
                for i in range(2):
                    nc.tensor.matmul(
                        st[:, i, q0:q0 + width],
                        kT[g][64 * i:64 * i + 64, kt * 128:(kt + 1) * 128],
                        qT[g][64 * i:64 * i + 64,
                              qb * QB + q0:qb * QB + q0 + width],
                        start=True,
                        stop=True,
                    )
                if fillers:
                    for f in fillers.get(kt, ()):
                        f()
                # AV lags 2 batches so the unit's first AV (which waits on
                # the previous unit's ot WAR) issues behind two score pairs.
                if len(pend) == 2:
                    emit_av(*pend.pop(0))
                pt = pt_pool.tile([128, 2, QB], bf16, tag="pt", name="pt")
                nc.scalar.activation(
                    pt[:, :, q0:q0 + width], st[:, :, q0:q0 + width],
                    func=EXP, scale=0.125,
                )
                if diag:
                    for i in range(2):
                        nc.vector.tensor_mul(
                            pt[:, i, q0:q0 + 128], pt[:, i, q0:q0 + 128],
                            mask_sb[:],
                        )
                pend.append((kt, pt, q0, width))
            for p in pend:
                emit_av(*p)

            # stream out unnormalized O^T + denominator row (host divides
            # + transposes); the casts already happened in emit_av.
            # (Tapering further into per-quarter DMAs was measured WORSE:
            # the extra sync triggers land in the ACT-paced final stretch.)
            lo = 256 if last else 0  # last unit's first half went out early
            nc.sync.dma_start(out=out_d[2 * g, :, qb, lo:QB],
                              in_=osb[:, 0, lo:QB])
            eng = nc.scalar if last else nc.sync
            eng.dma_start(out=out_d[2 * g + 1, :, qb, lo:QB],
                          in_=osb[:, 1, lo:QB])

        # ---- schedule ----
        # qb ASCENDING: unit (0, g0) needs only x block 0 (K chunk 0, Q
        # chunk 0, V tiles 0-3), so the exp stream starts as soon as the
        # first quarter of x^T lands.  Each unit's fillers project what the
        # NEXT units need, in x-arrival order.  An 8-warm bridge covers the
        # wq DMA wait.
        # NOTE: a narrow (N=128) first K projection starts the exp stream
        # earlier on paper but thins the PE right at the HAM window edge —
        # measured: MID re-throttle at ~16us and a HALF-CLOCK first unit.
        # Keep the prologue dense.  (No warm bridge between K and Q: wq
        # lands ~0.5us BEFORE the K projection ends in this ascending
        # schedule, so a bridge is pure critical-path overhead.)
        proj_qk(kT, wkT, 0, 0)
        proj_qk(qT, wqT, 0, 0)

        def F(*items):
            out = []
            for it in items:
                if it[0] == "v":
                    out.append(lambda t=it[1]: proj_v(t))
                elif it[0] == "k":
                    out.append(lambda g=it[1], c=it[2]: proj_qk(kT, wkT, g, c))
                else:
                    out.append(lambda g=it[1], c=it[2]: proj_qk(qT, wqT, g, c))
            return out

        fill_00 = {
            0: F(("v", 0)),
            1: F(("v", 1)),
            2: F(("v", 2), ("k", 1, 0)),
            3: F(("v", 3), ("q", 1, 0)),
        }
        fill_01 = {
            1: F(("k", 0, 1)),
            2: F(("q", 0, 1)),
            3: F(("v", 4)),
        }
        fill_10 = {
            0: F(("v", 5)),
            1: F(("k", 1, 1)),
            2: F(("q", 1, 1)),
            3: F(("v", 6)),
            4: F(("v", 7)),
            5: F(("k", 0, 2)),
            6: F(("v", 8)),
            7: F(("v", 9)),
        }
        fill_11 = {
            0: F(("q", 0, 2)),
            1: F(("v", 10)),
            2: F(("v", 11)),
            4: F(("q", 1, 2)),
            5: F(("k", 0, 3)),
            6: F(("v", 12)),
            7: F(("v", 13)),
        }
        fill_20 = {
            0: F(("v", 14)),
            1: F(("v", 15)),
            2: F(("q", 0, 3)),
        }
        attn_unit(0, 0, fillers=fill_00)
        attn_unit(0, 1, fillers=fill_01)
        attn_unit(1, 0, fillers=fill_10)
        attn_unit(1, 1, fillers=fill_11)
        attn_unit(2, 0, fillers=fill_20)
        # the last projections are saved as PREFILL for the otherwise
        # fillerless late units (see attn_unit)
        attn_unit(2, 1, prefill=F(("k", 1, 2)))
        attn_unit(3, 0, prefill=F(("k", 1, 3)))
        attn_unit(3, 1, prefill=F(("q", 1, 3)), last=True)

    nc.finalize()
    return nc


def _get_nc():
    if "nc" not in _CACHE:
        _CACHE["nc"] = _build_nc()
    return _CACHE["nc"]


def _make_cmask():
    # triangle: mask[p, f] = 1.0 iff p <= f
    p = np.arange(128)[:, None]
    f = np.arange(128)[None, :]
    return (p <= f).astype(ml_dtypes.bfloat16)


def _make_in_maps(x, Wq, Wk, Wv):
    bf = ml_dtypes.bfloat16
    cmask = _make_cmask()
    in_maps = []
    for c in range(N_CORES):
        b, hg = c // 2, c % 2
        r0 = hg * OUTW
        in_maps.append({
            "xt": np.ascontiguousarray(x[b].T).astype(bf),
            "wqt": np.ascontiguousarray(Wq[r0:r0 + OUTW].T).astype(bf),
            "wkt": np.ascontiguousarray(Wk[r0:r0 + OUTW].T).astype(bf),
            "wvt": np.ascontiguousarray(Wv[r0:r0 + OUTW].T).astype(bf),
            "cmask": cmask,
        })
    return in_maps


def _postprocess(results, B):
    """Host-side unshard: divide unnormalized O^T by the denominator row
    and transpose to natural [T, D] layout."""
    out = np.empty((B, T, D), dtype=np.float32)
    for c in range(N_CORES):
        b, hg = c // 2, c % 2
        r = results[c]["out"].astype(np.float32)  # [HG, 65, NQB, QB]
        r = r.reshape(HG, 65, T)
        for h in range(HG):
            blk = r[h, :64] / r[h, 64:65]  # [64, T]
            out[b, :, hg * OUTW + h * DH:hg * OUTW + (h + 1) * DH] = blk.T
    return out


def kernel(x, Wq, Wk, Wv):
    from concourse.bass_utils import run_bass_kernel_spmd

    nc = _get_nc()
    in_maps = _make_in_maps(x, Wq, Wk, Wv)
    res = run_bass_kernel_spmd(nc, in_maps, core_ids=list(range(N_CORES)))
    return _postprocess(res.results, x.shape[0])
